# revision 10
# baseline (speedup 1.0000x reference)
"""3x3 valid cross-correlation of a 4096x4096 fp32 image + scalar bias,
sharded row-wise across 8 TRN2 NeuronCores.

Strategy per core (512 output rows, 514 input rows incl. 2-row halo taken
host-side via overlapping slices -- no device collectives):
  - Row panels of 128 input rows -> 126 output rows (banded matmul):
    out[m, n] = sum_dc sum_dr w[dr, dc] * x[m+dr, n+dc]
    For each kernel column dc, a banded stationary matrix
    B_dc[k, m] = w[k-m, dc] (k-m in 0..2) gives
    (B_dc.T-free) matmul: psum[m, n] += sum_k B_dc[k, m] * x[k, n+dc].
    The 3 dc-matmuls accumulate into one PSUM bank; the column shift dc is
    folded into the moving-operand (rhs) free-dim offset.
  - x and w DRAM tensors are declared float32r so the DMA loads feed the
    PE directly (single-pass fp32 matmul, 1 cycle/row at N>=256) with no
    SBUF-to-SBUF converting copy stage.
  - All 5 panel loads (4x[128,4096] + tail [10,4096]) are issued up-front
    on the SP HWDGE ring with no backpressure; stores ride the ACT ring as
    whole [126,4094] panels so the two directions interleave on the 16
    SDMA engines.
  - Bias is fused into the PSUM->SBUF drains (ScalarE activation bias for
    one 2048-wide chunk, VectorE tensor_scalar_add for the other).
  - PE clock warmup: ~5us of throwaway fp32 matmuls (4 cyc/row) bridge
    the gap until the first panel's data lands, so real fp32r matmuls run
    at the ramped 2.4 GHz clock.
  - Last core overlaps core 6 by 2 rows so that all cores run an identical
    514-row program (4094 = 8*512 - 2).
"""

import numpy as np

import concourse.bacc as bacc
import concourse.mybir as mybir
from concourse import tile
from concourse.bass_utils import run_bass_kernel_spmd

H, W = 4096, 4096
KH, KW = 3, 3
OH, OW = H - KH + 1, W - KW + 1  # 4094, 4094
NCORES = 8
ROWS_PER_CORE = 512              # output rows computed per core
IN_ROWS = ROWS_PER_CORE + KH - 1  # 514 input rows per core
PANEL_OUT = 126                  # output rows per full 128-input-row panel
N_FULL_PANELS = 4                # 4 * 126 = 504
TAIL_OUT = ROWS_PER_CORE - N_FULL_PANELS * PANEL_OUT  # 8
TAIL_IN = TAIL_OUT + KH - 1      # 10
COLS_PER_MM = 512                # fp32 moving-operand / PSUM-bank max

_F32 = mybir.dt.float32
_F32R = mybir.dt.float32r

_PROGRAM_CACHE = None
last_results = None  # BassKernelResults of the most recent kernel() call


def _build_program():
    nc = bacc.Bacc(
        "TRN2", target_bir_lowering=False, debug=False, num_devices=NCORES
    )
    x = nc.dram_tensor("x", [IN_ROWS, W], _F32R, kind="ExternalInput")
    w = nc.dram_tensor("w", [128, KW * PANEL_OUT], _F32R, kind="ExternalInput")
    b = nc.dram_tensor("b", [128, 1], _F32, kind="ExternalInput")
    y = nc.dram_tensor("y", [ROWS_PER_CORE, OW], _F32, kind="ExternalOutput")

    with tile.TileContext(nc) as tc:
        with (
            tc.tile_pool(name="const", bufs=1) as cpool,
            tc.tile_pool(name="xp", bufs=5) as xpool,
            tc.tile_pool(name="op", bufs=3) as opool,
            tc.tile_pool(name="pp", bufs=2, space="PSUM") as ppool,
        ):
            # All x panel loads first: no dependencies, SP ring streams them
            # back-to-back while everything else spins up.
            xts = []
            for panel in range(N_FULL_PANELS + 1):
                r0 = PANEL_OUT * panel
                K = 128 if panel < N_FULL_PANELS else TAIL_IN
                xt = xpool.tile([128, W], _F32R)
                if panel < N_FULL_PANELS:
                    nc.sync.dma_start(xt[:K, :], x[r0 : r0 + K, :])
                else:
                    # HWDGE anchors a small (10-descriptor) DMA entirely on
                    # SDMA engine 0, which is already the straggler; SWDGE's
                    # partition swizzle spreads it over engines {0,2,4}.
                    nc.gpsimd.dma_start(xt[:K, :], x[r0 : r0 + K, :])
                xts.append(xt)

            # Weights + bias ride the ACT ring (idle until the first store)
            # so they don't queue behind 8.6 MB of x loads on the SP ring.
            wtr = cpool.tile([128, KW * PANEL_OUT], _F32R)
            nc.scalar.dma_start(wtr[:], w[:])
            bt = cpool.tile([128, 1], _F32)
            nc.scalar.dma_start(bt[:], b[:])

            # PE clock warmup: fp32 (two-pass, 4 cyc/row) matmuls on a memset
            # tile are long-running single instructions that keep the PE busy
            # until the first panel's data arrives (~3us of continuous PE work
            # ramps the clock to 2.4 GHz).
            wz = cpool.tile([128, COLS_PER_MM], _F32)
            nc.gpsimd.memset(wz[:], 0.0)
            pswarm = ppool.tile([128, COLS_PER_MM], _F32, tag="ps")
            for _ in range(4):
                nc.tensor.matmul(
                    pswarm[:126, :],
                    wz[:, :126],
                    wz[:, :],
                    start=True,
                    stop=True,
                )

            for panel in range(N_FULL_PANELS + 1):
                r0 = PANEL_OUT * panel
                K = 128 if panel < N_FULL_PANELS else TAIL_IN
                M = PANEL_OUT if panel < N_FULL_PANELS else TAIL_OUT
                xt = xts[panel]

                ot = opool.tile([128, OW], _F32)
                for c in range(2):
                    # One 4-bank PSUM tile per 2048-col half: each of the 4
                    # matmul groups lands in its own bank, then a single wide
                    # drain covers the half.
                    ps = ppool.tile([128, 4 * COLS_PER_MM], _F32, tag="ps")
                    s0 = c * 4 * COLS_PER_MM
                    sw = min(4 * COLS_PER_MM, OW - s0)  # 2048 / 2046
                    for jj in range(4):
                        c0 = s0 + jj * COLS_PER_MM
                        N = min(COLS_PER_MM, OW - c0)
                        lc0 = jj * COLS_PER_MM
                        for dc in range(KW):
                            nc.tensor.matmul(
                                ps[:M, lc0 : lc0 + N],
                                wtr[:K, dc * PANEL_OUT : dc * PANEL_OUT + M],
                                xt[:K, c0 + dc : c0 + dc + N],
                                start=(dc == 0),
                                stop=(dc == KW - 1),
                            )
                    # Drain PSUM on alternating engines so neither ScalarE
                    # nor VectorE becomes the bottleneck.
                    if c % 2 == 0:
                        nc.scalar.activation(
                            ot[:M, s0 : s0 + sw],
                            ps[:M, :sw],
                            mybir.ActivationFunctionType.Identity,
                            bias=bt[:M, :],
                        )
                    else:
                        nc.vector.tensor_scalar_add(
                            ot[:M, s0 : s0 + sw], ps[:M, :sw], bt[:M, :]
                        )
                # Stores as [M, 2048]-column chunks: HWDGE spreads these
                # evenly over SDMA engines 0-13 (whole-panel 16 KB rows skew
                # toward engines 0-7, and SWDGE's descriptor-ring traffic
                # slows engines 0-7 ~35%). Half rides ACT, half rides SP
                # (queued FIFO behind the loads, which is fine -- each store
                # is drain-ready before the ring reaches it), so two
                # descriptor streams keep the engines fed. 14 engines at
                # line rate still exceed the per-NC HBM cap, so engines
                # 14/15 idling during stores costs nothing.
                HALF = 4 * COLS_PER_MM
                if panel < N_FULL_PANELS:
                    nc.scalar.dma_start(y[r0 : r0 + M, :HALF], ot[:M, :HALF])
                    nc.sync.dma_start(y[r0 : r0 + M, HALF:OW], ot[:M, HALF:OW])
                else:
                    # Tail store: SWDGE spreads the 8 rows over engines {0,2}
                    # instead of anchoring them all on engine 0.
                    nc.gpsimd.dma_start(y[r0 : r0 + M, :], ot[:M, :OW])
    nc.compile()
    return nc


def _banded_weights(weight: np.ndarray) -> np.ndarray:
    """lhsT for each kernel column dc, laid out as [128, KW*PANEL_OUT].

    wT[k, dc*PANEL_OUT + m] = weight[k - m, dc] for 0 <= k - m < KH.
    The tail panel's [TAIL_IN, TAIL_OUT] banded matrix is the top-left
    block of the same layout, so one tensor serves both panel shapes.
    """
    wT = np.zeros((128, KW * PANEL_OUT), np.float32)
    m = np.arange(PANEL_OUT)
    for dc in range(KW):
        for d in range(KH):
            wT[m + d, dc * PANEL_OUT + m] = weight[d, dc]
    return wT


def _install_ntff_hook():
    """Shim antenv.axon_hooks so run_bass_kernel_spmd(trace=True) can find
    the axon NTFF profiling hook (the image's antenv lacks axon_hooks)."""
    import sys
    import types

    try:
        from antenv.axon_hooks import get_axon_ntff_profile_hook  # noqa: F401

        return
    except ImportError:
        pass
    import antenv
    from trn_agent_boot.trn_boot import _ntff_profile_via_ctypes

    hook = _ntff_profile_via_ctypes("/opt/axon/libaxon_pjrt.so")
    mod = types.ModuleType("antenv.axon_hooks")
    mod._hook = hook
    mod.set_axon_ntff_profile_hook = lambda h: setattr(mod, "_hook", h)
    mod.get_axon_ntff_profile_hook = lambda: mod._hook
    sys.modules["antenv.axon_hooks"] = mod
    antenv.axon_hooks = mod


def kernel(x, weight, bias, _trace=False, _trace_cores=None):
    global _PROGRAM_CACHE, last_results
    if _trace:
        _install_ntff_hook()
    x = np.ascontiguousarray(np.asarray(x, dtype=np.float32))
    weight = np.asarray(weight, dtype=np.float32)
    bias = np.asarray(bias, dtype=np.float32)

    if _PROGRAM_CACHE is None:
        _PROGRAM_CACHE = _build_program()
    nc = _PROGRAM_CACHE

    wT = _banded_weights(weight)
    bb = np.full((128, 1), bias[0], np.float32)

    in_maps = []
    for i in range(NCORES):
        r0 = i * ROWS_PER_CORE if i < NCORES - 1 else H - IN_ROWS
        in_maps.append(
            {"x": np.ascontiguousarray(x[r0 : r0 + IN_ROWS]), "w": wT, "b": bb}
        )

    kwargs = {}
    if _trace:
        kwargs["trace"] = True
        kwargs["trace_cores"] = (
            list(range(NCORES)) if _trace_cores is None else _trace_cores
        )
    res = run_bass_kernel_spmd(nc, in_maps, core_ids=list(range(NCORES)), **kwargs)
    last_results = res

    out = np.empty((OH, OW), np.float32)
    for i in range(NCORES - 1):
        out[i * ROWS_PER_CORE : (i + 1) * ROWS_PER_CORE] = res.results[i]["y"]
    tail_rows = OH - (NCORES - 1) * ROWS_PER_CORE  # 510
    out[(NCORES - 1) * ROWS_PER_CORE :] = res.results[-1]["y"][
        ROWS_PER_CORE - tail_rows :
    ]
    return out
